# revision 3
# baseline (speedup 1.0000x reference)
"""Trainium2 Bass kernel v2 for nn_GAT_LSTM: 3-layer GATv2 stack + LSTM + FC head.

Sharding (8 NeuronCores):
  Launch A: data-parallel over the 40 (B,T) graphs -> 5 graphs/core. Edges
            sorted by dst into 126-node blocks. Per chunk of 128 edges:
              S1  PE assembles v = xr[dst] + ee + xl[src] in PSUM. The edge
                  term ee = ea @ We is rank-2, folded into the gather matmul:
                  pne rows 126/127 carry ea0/ea1, xr tile rows 126/127 carry
                  the We rows.  xl[src] comes from bf16 indirect-DMA gathers
                  and is added via an identity matmul.
              S2  ACT prelu -> m;  DVE m*att (2x bf16) + grouped reduce -> lg
              S3  ACT exp at narrow width (20) -> p
              S4  POOL p*xl broadcast-mult (DVE for 1/8, balanced),
                  DVE den column copy, PE one-hot scatter accumulate.
            The chunk stream is software-pipelined with a 4-chunk skew so
            PE/ACT/DVE/POOL run concurrently; block b+1's pne/pen/gather
            loads prefetch one block ahead; the next layer's transform is
            emitted 6 iterations after each block tail so PE never waits.
  Launch B: LSTM w_ih [1024,64000] column-sharded 8-way (bf16), transposed
            matmul (out [128q, 40]) so PE cost is 3x lower; the LSTM biases
            ride along as an extra contraction row on core 0.
  Launch C: partial-gate reduce + LSTM scan + FC head, entirely in the
            transposed layout [128 q-part, ..., 4 batch] - no per-step
            transposes, wide-partition DVE/ACT ops.
"""
import sys

for _p in ("/opt/trn_rl_repo", "/root/.axon_site/_ro/trn_rl_repo"):
    if _p not in sys.path:
        sys.path.insert(0, _p)

from collections import defaultdict

import ml_dtypes
import numpy as np

import concourse.bass as bass
import concourse.bacc as bacc
import concourse.mybir as mybir
import concourse.tile as tile
from concourse import bass_utils
from concourse.masks import make_identity

F32 = mybir.dt.float32
BF16 = mybir.dt.bfloat16
FP16 = mybir.dt.float16
AF = mybir.ActivationFunctionType
OP = mybir.AluOpType
NPBF = ml_dtypes.bfloat16

P = 128
N = 2000
NPAD = 2048
BLK = 126                        # dst nodes per block (126 + 2 ea rows)
NBLK = (N + BLK - 1) // BLK      # 16
B, T = 4, 10
G = B * T                        # 40 graphs
NCORES = 8
GL = G // NCORES                 # 5 graphs per core
LAYERS = [(8, 4, 32), (128, 4, 32), (128, 4, 8)]   # (F_in, H, C)
HID = 256
GATE = 4 * HID                   # 1024
EMB = N * 32                     # 64000
KSL = EMB // NCORES              # 8000 w_ih rows per core
KPAD = ((KSL + 127) // 128) * 128  # 8064
GWS = [640, 640, 256]            # xld gather row widths per layer (padded)

_cache = {}


# ----------------------------------------------------------------------------
# host-side graph preprocessing (indexing + tiny edge-static math)
# ----------------------------------------------------------------------------
def prep_graph(edge_index, edge_attr):
    src = np.concatenate([np.asarray(edge_index[0], np.int64),
                          np.arange(N, dtype=np.int64)])
    dst = np.concatenate([np.asarray(edge_index[1], np.int64),
                          np.arange(N, dtype=np.int64)])
    ea = np.concatenate(
        [edge_attr, np.broadcast_to(np.asarray(edge_attr).mean(0), (N, 2))], axis=0
    ).astype(np.float32)

    order = np.argsort(dst, kind="stable")
    src_s, dst_s, ea_s = src[order], dst[order], ea[order]
    blk_of = dst_s // BLK

    chunks_per_blk = []
    src_pad, ldst_pad, valid_pad, ea_pad = [], [], [], []
    for b in range(NBLK):
        sel = blk_of == b
        e_src = src_s[sel]
        e_ldst = dst_s[sel] - b * BLK
        e_ea = ea_s[sel]
        ne = len(e_src)
        nch = max(1, (ne + P - 1) // P)
        pad = nch * P - ne
        # pad edges: src -> zero row NPAD-1, pne/pen columns all-zero
        src_pad.append(np.concatenate([e_src, np.full(pad, NPAD - 1, np.int64)]))
        ldst_pad.append(np.concatenate([e_ldst, np.zeros(pad, np.int64)]))
        valid_pad.append(np.concatenate([np.ones(ne, bool), np.zeros(pad, bool)]))
        ea_pad.append(np.concatenate([e_ea, np.zeros((pad, 2), np.float32)]))
        chunks_per_blk.append(nch)
    assert min(chunks_per_blk) >= 4, chunks_per_blk

    src_all = np.concatenate(src_pad)
    ldst_all = np.concatenate(ldst_pad)
    valid_all = np.concatenate(valid_pad)
    ea_all = np.concatenate(ea_pad)
    nch_total = sum(chunks_per_blk)

    # dma_gather wrapped-int16 index layout: flat edge i (= j*128+p) lives at
    # idx16[i % 16, i // 16]; replicated across the 8 gpsimd cores (128 rows).
    flat = src_all.astype(np.int16)                                    # [NCH*128]
    idx16 = flat.reshape(-1, 16).T.copy()                              # [16, NCH*8]
    idx16 = np.tile(idx16, (8, 1))                                     # [128, NCH*8]

    jj = np.repeat(np.arange(nch_total), P)
    ee_pos = np.tile(np.arange(P), nch_total)

    # pne_aug [j, n(128), e(128)]: one-hot dst gather + ea rows 126/127
    pne = np.zeros((nch_total, P, P), np.float32)
    pne[jj[valid_all], ldst_all[valid_all], ee_pos[valid_all]] = 1.0
    pne[jj[valid_all], BLK, ee_pos[valid_all]] = ea_all[valid_all, 0]
    pne[jj[valid_all], BLK + 1, ee_pos[valid_all]] = ea_all[valid_all, 1]
    # pen [j, e(128), n(128)]: one-hot scatter
    pen = np.zeros((nch_total, P, P), np.float32)
    pen[jj[valid_all], ee_pos[valid_all], ldst_all[valid_all]] = 1.0

    # merged device layout ppe [128, j, 256]: cols 0:128 = pne [n, j, e],
    # cols 128:256 = pen [e, j, n] (bf16)
    ppe_h = np.concatenate(
        [pne.transpose(1, 0, 2), pen.transpose(1, 0, 2)], axis=2)
    ppe_h = np.ascontiguousarray(ppe_h).astype(NPBF)
    return dict(
        chunks_per_blk=chunks_per_blk, nch_total=nch_total,
        idx16=idx16, ppe_h=ppe_h,
    )


def _bcast_const(vec, reps):
    """[F] -> [128, reps*F] partition-broadcast bf16 constant."""
    t = np.tile(np.asarray(vec, np.float32).reshape(-1), reps)
    return np.ascontiguousarray(np.broadcast_to(t, (P, t.size))).astype(NPBF)


# ----------------------------------------------------------------------------
# Launch A: GAT stack, 5 graphs per core, software-pipelined edge phase
# ----------------------------------------------------------------------------
def build_gat(chunks_per_blk, num_devices=NCORES):
    nch_total = sum(chunks_per_blk)
    maxch = max(chunks_per_blk)
    nc = bacc.Bacc("TRN2", target_bir_lowering=False, debug=False,
                   enable_asserts=False, num_devices=num_devices,
                   dynamic_dma_scratch_size=49152)
    xld0_d = nc.dram_tensor("xld0", [NPAD, GWS[0]], BF16, kind="ExternalInput")
    xr0_d = nc.dram_tensor("xr0", [NBLK * P, 640], BF16, kind="ExternalInput")
    w11_d = nc.dram_tensor("w11", [128, 256], BF16, kind="ExternalInput")
    w21_d = nc.dram_tensor("w21", [128, 64], BF16, kind="ExternalInput")
    ppe_d = nc.dram_tensor("ppe", [P, nch_total, 256], BF16, kind="ExternalInput")
    idx16_d = nc.dram_tensor("idx16", [P, nch_total * 8], mybir.dt.int16,
                             kind="ExternalInput")
    wet_d = [nc.dram_tensor(f"wet{l}", [32, GL * LAYERS[l][1] * LAYERS[l][2]],
                            BF16, kind="ExternalInput") for l in range(3)]
    attb_d = [nc.dram_tensor(f"attb{l}", [P, GL * LAYERS[l][1] * LAYERS[l][2]],
                             BF16, kind="ExternalInput") for l in range(3)]
    biasb_d = [nc.dram_tensor(f"biasb{l}", [P, GL * LAYERS[l][1] * LAYERS[l][2]],
                              FP16, kind="ExternalInput") for l in range(3)]
    emb_d = nc.dram_tensor("emb", [GL, EMB], BF16, kind="ExternalOutput")

    ws_d = [None, w11_d, w21_d]
    # chunk global list: (block, local j, first, last)
    chunk_info = []
    for b in range(NBLK):
        for jl in range(chunks_per_blk[b]):
            chunk_info.append((b, jl, jl == 0, jl == chunks_per_blk[b] - 1))
    NCH = len(chunk_info)
    blk_first_jg = np.cumsum([0] + chunks_per_blk).tolist()

    with tile.TileContext(nc) as tc:
        with (
            tc.tile_pool(name="const", bufs=1) as cp,
            tc.tile_pool(name="xr", bufs=1) as xrp,
            tc.tile_pool(name="oblk", bufs=1) as obp,
            tc.tile_pool(name="gall", bufs=3) as gp,
            tc.tile_pool(name="edge", bufs=6) as ep,
            tc.tile_pool(name="work", bufs=6) as wp,
            tc.tile_pool(name="stage", bufs=2) as stg,
            tc.tile_pool(name="stream", bufs=3) as strm,
            tc.tile_pool(name="psv", bufs=2, space="PSUM") as pv,
            tc.tile_pool(name="psacc", bufs=2, space="PSUM") as pacc,
            tc.tile_pool(name="dram", bufs=1, space="DRAM") as dp,
        ):
            ident = cp.tile([P, P], F32)
            make_identity(nc, ident[:])
            identb = cp.tile([P, P], BF16)
            nc.vector.tensor_copy(out=identb[:], in_=ident[:])
            idx16_t = cp.tile([P, nch_total * 8], mybir.dt.int16)
            nc.sync.dma_start(out=idx16_t[:], in_=idx16_d[:, :])
            zer = cp.tile([P, 640], BF16)
            nc.vector.memset(zer[:], 0.0)
            w_t = [None,
                   cp.tile([128, 256], BF16, tag="w1", name="w1t"),
                   cp.tile([128, 64], BF16, tag="w2", name="w2t")]
            attb_t, biasb_t, wet_t = [], [], []
            for l in range(3):
                if ws_d[l] is not None:
                    nc.sync.dma_start(out=w_t[l][:], in_=ws_d[l][:, :])
                fo = LAYERS[l][1] * LAYERS[l][2]
                at = cp.tile([P, GL * fo], BF16, tag=f"attb{l}", name=f"attb{l}t")
                bt = cp.tile([P, GL * fo], FP16, tag=f"biasb{l}", name=f"biasb{l}t")
                et = cp.tile([32, GL * fo], BF16, tag=f"wet{l}", name=f"wet{l}t")
                nc.sync.dma_start(out=at[:], in_=attb_d[l][:, :])
                nc.sync.dma_start(out=bt[:], in_=biasb_d[l][:, :])
                nc.sync.dma_start(out=et[:], in_=wet_d[l][:, :])
                attb_t.append(at)
                biasb_t.append(bt)
                wet_t.append(et)

            xld = [xld0_d,
                   dp.tile([NPAD, GWS[1]], BF16, name="xld1"),
                   dp.tile([NPAD, GWS[2]], BF16, name="xld2")]
            # zero the pad rows (gathers of pad edges read row NPAD-1);
            # xld0 comes pre-zeroed from the host
            for l in (1, 2):
                nc.sync.dma_start(out=xld[l][NPAD - 32:NPAD, :],
                                  in_=zer[:32, :GWS[l]])

            o_blocks = [obp.tile([P, GL, 128], BF16, tag=f"o{b}", name=f"o{b}")
                        for b in range(NBLK)]
            xr_blocks = [xrp.tile([P, 640], BF16, tag=f"xr{b}", name=f"xr{b}")
                         for b in range(NBLK)]
            # one-time: make o_blocks pad rows finite for the DMA transposes
            for b in range(NBLK):
                nc.vector.memset(o_blocks[b][96:128, :, :], 0.0)

            txf_st = {}

            def txf_mm(l, b, g0, g1, pool_tag):
                """Transform matmuls for graphs [g0, g1) of block b, layer l."""
                fin, hh, cc = LAYERS[l]
                fo = hh * cc
                n0 = b * BLK
                if pool_tag == "acc":
                    ps = pacc.tile([P, 1024], F32, tag="acc")
                else:
                    ps = pv.tile([P, 1024], F32, tag="v")
                if g0 == 0:
                    # We rows for the ee fold (rows 126/127 via 32-aligned
                    # write; rows 96..125 are re-written by the xr copies)
                    nc.vector.tensor_copy(out=xr_blocks[b][96:128, :GL * fo],
                                          in_=wet_t[l][:, :GL * fo])
                hT5 = txf_st[("hT5", b)]
                for g in range(g0, g1):
                    lhsT = hT5[:, g, 0:126]
                    nc.tensor.matmul(
                        out=ps[:126, (g - g0) * 2 * fo:(g - g0 + 1) * 2 * fo],
                        lhsT=lhsT, rhs=w_t[l][:fin, :],
                        start=True, stop=True)
                txf_st[("ps", b, g0)] = ps

            def txf_copy(l, b, g0, g1, eng_xl, eng_xr):
                """PSUM -> xl_st / xr_blocks copies for graphs [g0, g1)."""
                fin, hh, cc = LAYERS[l]
                fo = hh * cc
                ps = txf_st.pop(("ps", b, g0))
                if ("xl", b) not in txf_st:
                    txf_st[("xl", b)] = stg.tile([P, 640], BF16, tag="xl_st",
                                                 name="xl_st")
                xl_st = txf_st[("xl", b)]
                psv = ps[:126, :(g1 - g0) * 2 * fo].rearrange(
                    "p (g t f) -> p g t f", t=2, f=fo)
                eng_xl(out=xl_st[:126, g0 * fo:g1 * fo].rearrange(
                           "p (g f) -> p g f", f=fo), in_=psv[:, :, 0, :])
                eng_xr(out=xr_blocks[b][:126, g0 * fo:g1 * fo].rearrange(
                           "p (g f) -> p g f", f=fo), in_=psv[:, :, 1, :])

            def txf_store(l, b):
                """We rows + xld DRAM writes for block b of layer l."""
                fin, hh, cc = LAYERS[l]
                fo = hh * cc
                gfo = GL * fo
                n0 = b * BLK
                xl_st = txf_st.pop(("xl", b))
                txf_st.pop(("hT5", b), None)
                nc.sync.dma_start(out=xld[l][n0:n0 + 126, :gfo],
                                  in_=xl_st[:126, :gfo])
                if GWS[l] > gfo:
                    nc.sync.dma_start(out=xld[l][n0:n0 + 126, gfo:],
                                      in_=zer[:126, :GWS[l] - gfo])

            def act_copy(out, in_):
                nc.scalar.activation(out=out, in_=in_, func=AF.Copy)

            def txf_transposes(b):
                # ACT HWDGE ring: the wait on o_blocks (an ACT write) is
                # satisfied in ACT program order -> no SP head-of-line block.
                hT5 = stg.tile([P, GL, 128], BF16, tag="hT5", name="hT5")
                for g in range(GL):
                    nc.sync.dma_start(out=hT5[:, g, :],
                                      in_=o_blocks[b][:, g, :],
                                      transpose=True)
                txf_st[("hT5", b)] = hT5

            # ---- layer 0 xr tiles come straight from the host ----
            for b in range(NBLK):
                nc.sync.dma_start(out=xr_blocks[b][:, :],
                                  in_=xr0_d[b * P:(b + 1) * P, :])

            # ---- edge phases, pipelined; transform l+1 folded in ----
            for l in range(3):
                fin, hh, cc = LAYERS[l]
                fo = hh * cc
                gfo = GL * fo
                ghh = GL * hh
                wpx = gfo + ghh
                GW = GWS[l]
                spans = [(s, min(s + 512, gfo)) for s in range(0, gfo, 512)]
                spans_px = [(s, min(s + 512, wpx)) for s in range(0, wpx, 512)]

                ppe_tiles = {}
                gall_tiles = {}
                loaded = [1]
                ps_acc_tiles = {}
                st = {}
                sched = defaultdict(list)

                def load_block(b, l=l, ppe_tiles=ppe_tiles,
                               gall_tiles=gall_tiles, GW=GW):
                    nch = chunks_per_blk[b]
                    j0 = blk_first_jg[b]
                    ppe_t = strm.tile([P, maxch, 256], BF16, tag="ppe")
                    nc.sync.dma_start(out=ppe_t[:, :nch, :],
                                      in_=ppe_d[:, j0:j0 + nch, :])
                    g_all = gp.tile([P, maxch, GW], BF16, tag="gall")
                    nc.gpsimd.dma_gather(
                        out_ap=g_all[:, 0:nch, :],
                        in_ap=xld[l][:, :],
                        idxs_ap=idx16_t[:, j0 * 8:(j0 + nch) * 8],
                        num_idxs=nch * P, num_idxs_reg=nch * P,
                        elem_size=GW, single_packet=False)
                    ppe_tiles[b] = ppe_t
                    gall_tiles[b] = g_all

                def s1(k):
                    b, jl, first, last = chunk_info[k]
                    if first and b == 0:
                        load_block(0)
                    ps_v = pv.tile([P, 1024], F32, tag="v")
                    ppe_t = ppe_tiles[b]
                    g_all = gall_tiles[b]
                    for (s0, s1_) in spans:
                        nc.tensor.matmul(out=ps_v[:, s0:s1_],
                                         lhsT=ppe_t[:, jl, 0:128],
                                         rhs=xr_blocks[b][:, s0:s1_],
                                         start=True, stop=False)
                        nc.tensor.matmul(out=ps_v[:, s0:s1_],
                                         lhsT=identb[:],
                                         rhs=g_all[:, jl, s0:s1_],
                                         start=False, stop=True)
                    st[("v", k)] = ps_v

                def a_prelu(k):
                    ps_v = st.pop(("v", k))
                    m_t = ep.tile([P, 640], BF16, tag="m")
                    nc.scalar.activation(out=m_t[:, :gfo], in_=ps_v[:, :gfo],
                                         func=AF.Prelu, alpha=0.2)
                    st[("m", k)] = m_t

                def v_amtr(k):
                    m_t = st.pop(("m", k))
                    am = wp.tile([P, 640], BF16, tag="am")
                    nc.vector.tensor_tensor(out=am[:, :gfo], in0=m_t[:, :gfo],
                                            in1=attb_t[l][:], op=OP.mult)
                    lg = ep.tile([P, ghh], F32, tag="lg")
                    nc.vector.tensor_reduce(
                        out=lg[:],
                        in_=am[:, :gfo].rearrange("p (t c) -> p t c", c=cc),
                        axis=mybir.AxisListType.X, op=OP.add)
                    st[("lg", k)] = lg

                def a_exp(k):
                    lg = st.pop(("lg", k))
                    # exp written straight into the pxl den columns
                    pxl = ep.tile([P, 680], BF16, tag="pxl", name="pxl")
                    nc.scalar.activation(out=pxl[:, gfo:wpx], in_=lg[:],
                                         func=AF.Exp)
                    st[("pxl", k)] = pxl

                def s_pxl(k):
                    b, jl, first, last = chunk_info[k]
                    if first:
                        while loaded[0] <= b + 2 and loaded[0] < NBLK:
                            load_block(loaded[0])
                            loaded[0] += 1
                    pxl = st[("pxl", k)]
                    g_all = gall_tiles[b]
                    eng = nc.gpsimd if (k % 16 and l < 2) else nc.vector
                    eng.tensor_tensor(
                        out=pxl[:, :gfo].rearrange("p (t c) -> p t c", c=cc),
                        in0=g_all[:, jl, :gfo].rearrange("p (t c) -> p t c", c=cc),
                        in1=pxl[:, gfo:wpx].rearrange("p (t u) -> p t u", u=1)
                            .to_broadcast([P, ghh, cc]),
                        op=OP.mult)

                def s4(k, i):
                    b, jl, first, last = chunk_info[k]
                    pxl = st.pop(("pxl", k))
                    ppe_t = ppe_tiles[b]
                    if first:
                        ps_acc_tiles[b] = pacc.tile([P, 1024], F32, tag="acc",
                                                    name=f"acc{b}")
                    ps_acc = ps_acc_tiles[b]
                    for (s0, s1_) in spans_px:
                        nc.tensor.matmul(out=ps_acc[:, s0:s1_],
                                         lhsT=ppe_t[:, jl, 128:256],
                                         rhs=pxl[:, s0:s1_],
                                         start=first, stop=last)
                    if last:
                        offs = ((1, 1, 1, 2, 2, 2, 3, 3, 4) if b >= NBLK - 2
                                else (2, 3, 4, 5, 6, 7, 8, 9, 10))
                        sched[i + offs[0]].append(lambda b=b: tail_den(b))
                        sched[i + offs[1]].append(lambda b=b: tail_cp(b))
                        sched[i + offs[2]].append(lambda b=b: tail_mult(b))
                        sched[i + offs[3]].append(lambda b=b: tail_bias(b))
                        sched[i + offs[4]].append(lambda b=b: tail_relu(b))
                        if l < 2:
                            sched[i + offs[5]].append(
                                lambda b=b: txf_transposes(b))
                            sched[i + offs[6]].append(
                                lambda b=b: txf_mm(l + 1, b, 0, 4, "acc"))
                            sched[i + offs[7]].append(
                                lambda b=b: (txf_copy(l + 1, b, 0, 4, act_copy,
                                                      act_copy),
                                             txf_mm(l + 1, b, 4, 5, "v")))
                            sched[i + offs[8]].append(
                                lambda b=b: (txf_copy(l + 1, b, 4, 5, act_copy,
                                                      act_copy),
                                             txf_store(l + 1, b)))

                def tail_den(b):
                    ps_acc = ps_acc_tiles[b]
                    den_t = wp.tile([P, ghh], F32, tag="den")
                    nc.vector.tensor_scalar_add(out=den_t[:126, :],
                                                in0=ps_acc[:126, gfo:wpx],
                                                scalar1=1e-4)
                    rec_t = wp.tile([P, ghh], F32, tag="rec")
                    nc.vector.reciprocal(out=rec_t[:126, :], in_=den_t[:126, :])
                    st[("rec", b)] = rec_t

                def tail_cp(b):
                    # PSUM -> bf16 SBUF copies so the divide runs at DVE 2x
                    ps_acc = ps_acc_tiles.pop(b)
                    rec_t = st.pop(("rec", b))
                    acc_sb = wp.tile([P, 640], FP16, tag="acc_sb")
                    nc.scalar.activation(out=acc_sb[:126, :gfo],
                                         in_=ps_acc[:126, :gfo], func=AF.Copy)
                    rec_e = wp.tile([P, 640], FP16, tag="rec_e")
                    nc.scalar.activation(
                        out=rec_e[:126, :gfo].rearrange("p (t c) -> p t c", c=cc),
                        in_=rec_t[:126, :].rearrange("p (t u) -> p t u", u=1)
                            .to_broadcast([126, ghh, cc]),
                        func=AF.Copy)
                    st[("acc", b)] = acc_sb
                    st[("rece", b)] = rec_e

                def tail_mult(b):
                    acc_sb = st.pop(("acc", b))
                    rec_e = st.pop(("rece", b))
                    o_t = wp.tile([P, 640], FP16, tag="o_t")
                    nc.vector.tensor_tensor(out=o_t[:126, :gfo],
                                            in0=acc_sb[:126, :gfo],
                                            in1=rec_e[:126, :gfo], op=OP.mult)
                    st[("ot", b)] = o_t

                def tail_bias(b):
                    o_t = st[("ot", b)]
                    nc.vector.tensor_tensor(out=o_t[:126, :gfo],
                                            in0=o_t[:126, :gfo],
                                            in1=biasb_t[l][:126, :], op=OP.add)

                def tail_relu(b):
                    o_t = st.pop(("ot", b))
                    if l < 2:
                        nc.scalar.activation(
                            out=o_blocks[b][:126, :, :].rearrange(
                                "p g f -> p (g f)"),
                            in_=o_t[:126, :gfo], func=AF.Relu)
                    else:
                        o2 = stg.tile([P, GL, 32], BF16, tag="o2")
                        nc.scalar.activation(
                            out=o2[:126, :, :].rearrange("p g f -> p (g f)"),
                            in_=o_t[:126, :gfo], func=AF.Relu)
                        rows = min(126, N - b * BLK)
                        nc.sync.dma_start(
                            out=emb_d[:, :].rearrange("g (n c) -> g n c", c=32)[
                                :, b * BLK:b * BLK + rows, :]
                                .rearrange("g p c -> p g c"),
                            in_=o2[:rows, :, :])

                # skewed emission: iteration i runs S1(i), exp(i-3), prelu(i-1),
                # am/TR(i-2), pxl(i-4), S4(i-5); block tails + next-layer
                # transforms are spread over iterations i+1 .. i+10.
                for i in range(NCH + 16):
                    if i < NCH:
                        s1(i)
                    if 3 <= i < NCH + 3:
                        a_exp(i - 3)
                    if 1 <= i < NCH + 1:
                        a_prelu(i - 1)
                    if 2 <= i < NCH + 2:
                        v_amtr(i - 2)
                    if 4 <= i < NCH + 4:
                        s_pxl(i - 4)
                    if 5 <= i < NCH + 5:
                        s4(i - 5, i)
                    for fn in sched.pop(i, []):
                        fn()
    nc.compile()
    return nc


# ----------------------------------------------------------------------------
# Launch B: partial LSTM input-gate products, transposed (out [128q, 8, 40])
# ----------------------------------------------------------------------------
def build_gates():
    nc = bacc.Bacc("TRN2", target_bir_lowering=False, debug=False,
                   enable_asserts=False, num_devices=NCORES)
    embT_d = nc.dram_tensor("embT", [KPAD, G], BF16, kind="ExternalInput")
    wT_d = nc.dram_tensor("wT", [KPAD, GATE], BF16, kind="ExternalInput")
    part_d = nc.dram_tensor("part", [P, 8 * G], F32, kind="ExternalOutput")
    KCH = KPAD // P          # 63
    KB = 3                   # k-tiles per DMA chunk (63 = 21*3)
    with tile.TileContext(nc) as tc:
        with (
            tc.tile_pool(name="sb", bufs=1) as sp,
            tc.tile_pool(name="wstream", bufs=3) as wsp,
            tc.tile_pool(name="ps", bufs=1, space="PSUM") as pp,
        ):
            embT_t = sp.tile([P, KCH, G], BF16)
            nc.sync.dma_start(out=embT_t[:],
                              in_=embT_d[:, :].rearrange("(k p) g -> p k g", p=P))
            # one PSUM bank per qt so the 8 k-interleaved accumulation
            # groups live in distinct zero regions
            ps = pp.tile([P, 8, 512], F32)
            for k0 in range(0, KCH, KB):
                w_t = wsp.tile([P, KB, GATE], BF16, tag="w")
                nc.sync.dma_start(
                    out=w_t[:],
                    in_=wT_d[k0 * P:(k0 + KB) * P, :].rearrange(
                        "(k p) q -> p k q", p=P))
                for dk in range(KB):
                    k = k0 + dk
                    for qt in range(8):
                        nc.tensor.matmul(out=ps[:, qt, :G],
                                         lhsT=w_t[:, dk, qt * P:(qt + 1) * P],
                                         rhs=embT_t[:, k, :],
                                         start=(k == 0), stop=(k == KCH - 1))
            out_t = sp.tile([P, 8 * G], F32)
            nc.vector.tensor_copy(out=out_t[:].rearrange("p (a g) -> p a g", g=G),
                                  in_=ps[:, :, :G])
            nc.sync.dma_start(out=part_d[:, :], in_=out_t[:])
    nc.compile()
    return nc


# ----------------------------------------------------------------------------
# Launch C: reduce partials + LSTM scan + FC head (transposed layout)
# ----------------------------------------------------------------------------
def build_scan():
    nc = bacc.Bacc("TRN2", target_bir_lowering=False, debug=False,
                   enable_asserts=False, num_devices=NCORES)
    # parts pre-laid-out host-side to [128, qt(8), g(40), core(8)]
    parts_d = nc.dram_tensor("parts", [P, 8 * G * NCORES], F32,
                             kind="ExternalInput")
    whhT_d = nc.dram_tensor("whhT", [P, 2 * GATE], BF16, kind="ExternalInput")
    fc1w_d = nc.dram_tensor("fc1w", [P, 2 * 512], BF16, kind="ExternalInput")
    fc1b_d = nc.dram_tensor("fc1b", [P, 4], F32, kind="ExternalInput")
    fc2w_d = nc.dram_tensor("fc2w", [P, 4], BF16, kind="ExternalInput")
    fc2b_d = nc.dram_tensor("fc2b", [B, 1], F32, kind="ExternalInput")
    out_d = nc.dram_tensor("out", [B, 1], F32, kind="ExternalOutput")
    with tile.TileContext(nc) as tc:
        with (
            tc.tile_pool(name="sb", bufs=1) as sp,
            tc.tile_pool(name="wk", bufs=2) as wk,
            tc.tile_pool(name="ps", bufs=2, space="PSUM") as pp,
        ):
            parts_t = sp.tile([P, 8 * G, NCORES], F32)
            nc.sync.dma_start(
                out=parts_t[:],
                in_=parts_d[:, :].rearrange("p (q r) -> p q r", r=NCORES))
            whhT_t = sp.tile([P, 2, GATE], BF16)
            nc.sync.dma_start(out=whhT_t[:],
                              in_=whhT_d[:, :].rearrange("p (k q) -> p k q", k=2))
            fc1w_t = sp.tile([P, 2, 512], BF16)
            nc.sync.dma_start(out=fc1w_t[:],
                              in_=fc1w_d[:, :].rearrange("p (k q) -> p k q", k=2))
            fc1b_t = sp.tile([P, 4], F32)
            nc.sync.dma_start(out=fc1b_t[:], in_=fc1b_d[:, :])
            fc2w_t = sp.tile([P, 4], BF16)
            nc.sync.dma_start(out=fc2w_t[:], in_=fc2w_d[:, :])
            fc2b_t = sp.tile([B, 1], F32)
            nc.sync.dma_start(out=fc2b_t[:], in_=fc2b_d[:, :])

            # gihT [128, qt(8), g(40)] = sum over cores (biases folded in B)
            gih_t = sp.tile([P, 8, G], F32)
            nc.vector.tensor_reduce(out=gih_t[:].rearrange("p a g -> p (a g)"),
                                    in_=parts_t[:],
                                    axis=mybir.AxisListType.X, op=OP.add)
            gih_v = gih_t[:].rearrange("p a (g tt) -> p a g tt", tt=T)

            hT = sp.tile([P, 2, B], BF16, tag="h")
            cT = sp.tile([P, 2, B], F32, tag="c")
            nc.vector.memset(hT[:], 0.0)
            nc.vector.memset(cT[:], 0.0)

            for t in range(T):
                ps_g = pp.tile([P, 8, B], F32, tag="g")
                for qt in range(8):
                    for kt in range(2):
                        nc.tensor.matmul(
                            out=ps_g[:, qt, :],
                            lhsT=whhT_t[:, kt, qt * P:(qt + 1) * P],
                            rhs=hT[:, kt, :],
                            start=(kt == 0), stop=(kt == 1))
                g_t = wk.tile([P, 8, B], F32, tag="gt")
                nc.vector.tensor_tensor(
                    out=g_t[:], in0=ps_g[:],
                    in1=gih_v[:, :, :, t], op=OP.add)
                # gate order is host-permuted to [i, f, o, g]
                sif = wk.tile([P, 6, B], F32, tag="sif")
                nc.scalar.activation(out=sif[:], in_=g_t[:, 0:6, :],
                                     func=AF.Sigmoid)
                tg = wk.tile([P, 2, B], F32, tag="tg")
                nc.scalar.activation(out=tg[:], in_=g_t[:, 6:8, :], func=AF.Tanh)
                c_new = sp.tile([P, 2, B], F32, tag=f"c{t}")
                nc.vector.tensor_tensor(out=c_new[:], in0=sif[:, 2:4, :],
                                        in1=cT[:], op=OP.mult)
                it = wk.tile([P, 2, B], F32, tag="it")
                nc.vector.tensor_tensor(out=it[:], in0=sif[:, 0:2, :],
                                        in1=tg[:], op=OP.mult)
                nc.vector.tensor_tensor(out=c_new[:], in0=c_new[:], in1=it[:],
                                        op=OP.add)
                tc_t = wk.tile([P, 2, B], F32, tag="tc")
                nc.scalar.activation(out=tc_t[:], in_=c_new[:], func=AF.Tanh)
                h_new = sp.tile([P, 2, B], BF16, tag=f"h{t}")
                nc.vector.tensor_tensor(out=h_new[:], in0=sif[:, 4:6, :],
                                        in1=tc_t[:], op=OP.mult)
                cT = c_new
                hT = h_new

            lastT = sp.tile([P, 2, B], BF16, tag="lastT")
            nc.scalar.activation(out=lastT[:], in_=hT[:], func=AF.Relu)
            ps_h = pp.tile([P, 4, B], F32, tag="ph")
            for qt in range(4):
                for kt in range(2):
                    nc.tensor.matmul(out=ps_h[:, qt, :],
                                     lhsT=fc1w_t[:, kt, qt * P:(qt + 1) * P],
                                     rhs=lastT[:, kt, :],
                                     start=(kt == 0), stop=(kt == 1))
            hidf = wk.tile([P, 4, B], F32, tag="hidf")
            nc.vector.tensor_tensor(
                out=hidf[:], in0=ps_h[:],
                in1=fc1b_t[:].rearrange("p (q u) -> p q u", u=1)
                    .to_broadcast([P, 4, B]),
                op=OP.add)
            hidT = sp.tile([P, 4, B], BF16, tag="hidT")
            nc.scalar.activation(out=hidT[:], in_=hidf[:], func=AF.Relu)
            ps_o = pp.tile([B, 1], F32, tag="po")
            for kt in range(4):
                nc.tensor.matmul(out=ps_o[:, :], lhsT=hidT[:, kt, :],
                                 rhs=fc2w_t[:, kt:kt + 1], start=(kt == 0),
                                 stop=(kt == 3))
            o_t = wk.tile([B, 1], F32, tag="o")
            nc.vector.tensor_tensor(out=o_t[:], in0=ps_o[:, :], in1=fc2b_t[:],
                                    op=OP.add)
            nc.sync.dma_start(out=out_d[:, :], in_=o_t[:])
    nc.compile()
    return nc


# ----------------------------------------------------------------------------
# kernel entry
# ----------------------------------------------------------------------------
def kernel(**inputs):
    x = np.asarray(inputs["x"], np.float32)
    edge_index = np.asarray(inputs["edge_index"])
    edge_attr = np.asarray(inputs["edge_attr"], np.float32)

    gp = prep_graph(edge_index, edge_attr)
    key = tuple(gp["chunks_per_blk"])
    if ("A", key) not in _cache:
        _cache[("A", key)] = build_gat(gp["chunks_per_blk"])
    if "B" not in _cache:
        _cache["B"] = build_gates()
    if "C" not in _cache:
        _cache["C"] = build_scan()
    ncA, ncB, ncC = _cache[("A", key)], _cache["B"], _cache["C"]

    # ---- Launch A inputs ----
    xg = x.reshape(G, N, 8)
    w01f = np.concatenate([inputs["w_l0"], inputs["w_r0"]], 1).astype(np.float32)
    w11 = np.concatenate([inputs["w_l1"], inputs["w_r1"]], 1).astype(NPBF)
    w21 = np.concatenate([inputs["w_l2"], inputs["w_r2"]], 1).astype(NPBF)
    atts = [inputs["att0"], inputs["att1"], inputs["att2"]]
    biases = [inputs["b0"], inputs["b1"], inputs["b2"]]
    wes = [inputs["w_e0"], inputs["w_e1"], inputs["w_e2"]]
    common = {
        "w11": w11, "w21": w21,
        "ppe": gp["ppe_h"], "idx16": gp["idx16"],
    }
    for l in range(3):
        common[f"attb{l}"] = _bcast_const(atts[l], GL)
        common[f"biasb{l}"] = _bcast_const(biases[l], GL).astype(np.float16)
        fo = LAYERS[l][1] * LAYERS[l][2]
        wet = np.zeros((32, GL * fo), np.float32)
        wet[30:32] = np.tile(np.asarray(wes[l], np.float32), (1, GL))
        common[f"wet{l}"] = np.ascontiguousarray(wet).astype(NPBF)
    in_maps = []
    for c in range(NCORES):
        m = dict(common)
        # layer-0 transform on host: xlr0 [GL, N, 256] = x @ [Wl0 | Wr0]
        xlr0 = xg[c * GL:(c + 1) * GL].astype(np.float32) @ w01f
        xld0 = np.zeros((NPAD, 640), np.float32)
        xld0[:N] = xlr0[:, :, :128].transpose(1, 0, 2).reshape(N, 640)
        xr0 = np.zeros((NBLK * P, 640), np.float32)
        xrn = xlr0[:, :, 128:].transpose(1, 0, 2).reshape(N, 640)
        for b in range(NBLK):
            rows = min(126, N - b * BLK)
            xr0[b * P:b * P + rows] = xrn[b * BLK:b * BLK + rows]
            xr0[b * P + 126:b * P + 128] = np.tile(
                np.asarray(wes[0], np.float32), (1, GL))
        m["xld0"] = xld0.astype(NPBF)
        m["xr0"] = xr0.astype(NPBF)
        in_maps.append(m)
    resA = bass_utils.run_bass_kernel_spmd(ncA, in_maps, core_ids=list(range(NCORES)))
    emb_all = np.concatenate(
        [np.asarray(resA.results[c]["emb"]) for c in range(NCORES)], 0)  # bf16 [G, EMB]

    # ---- Launch B ----
    embT_full = np.ascontiguousarray(emb_all.T)          # [64000, 40] bf16
    # permute LSTM gate order [i, f, g, o] -> [i, f, o, g] so the scan's
    # sigmoids are contiguous
    gperm = np.concatenate([np.arange(0, 512), np.arange(768, 1024),
                            np.arange(512, 768)])
    wT_full = np.ascontiguousarray(
        np.asarray(inputs["w_ih"], np.float32)[gperm].T).astype(NPBF)
    bias_row = (np.asarray(inputs["b_ih"], np.float32)
                + np.asarray(inputs["b_hh"], np.float32))[gperm].astype(NPBF)
    in_mapsB = []
    for c in range(NCORES):
        embT = np.zeros((KPAD, G), NPBF)
        wT = np.zeros((KPAD, GATE), NPBF)
        embT[:KSL] = embT_full[c * KSL:(c + 1) * KSL]
        wT[:KSL] = wT_full[c * KSL:(c + 1) * KSL]
        if c == 0:
            embT[KSL, :] = NPBF(1.0)
            wT[KSL, :] = bias_row
        in_mapsB.append({"embT": embT, "wT": wT})
    resB = bass_utils.run_bass_kernel_spmd(ncB, in_mapsB, core_ids=list(range(NCORES)))
    parts = np.stack([np.asarray(resB.results[c]["part"], np.float32)
                      for c in range(NCORES)], -1)       # [128, 320, 8]

    # ---- Launch C ----
    parts_pre = np.ascontiguousarray(parts.reshape(P, 8 * G * NCORES))
    whhT = np.asarray(inputs["w_hh"], np.float32)[gperm].T  # [256, 1024]
    whhT_pre = np.ascontiguousarray(
        whhT.reshape(2, P, GATE).transpose(1, 0, 2).reshape(P, 2 * GATE)
    ).astype(NPBF)
    fc1w = np.asarray(inputs["fc1_w"], np.float32)       # [256, 512]
    fc1w_pre = np.ascontiguousarray(
        fc1w.reshape(2, P, 512).transpose(1, 0, 2).reshape(P, 2 * 512)
    ).astype(NPBF)
    fc1b_pre = np.ascontiguousarray(
        np.asarray(inputs["fc1_b"], np.float32).reshape(4, P).T)
    fc2w_pre = np.ascontiguousarray(
        np.asarray(inputs["fc2_w"], np.float32).reshape(4, P).T).astype(NPBF)
    fc2b_pre = np.broadcast_to(
        np.asarray(inputs["fc2_b"], np.float32), (B, 1)).copy()
    in_mapsC = [{
        "parts": parts_pre,
        "whhT": whhT_pre,
        "fc1w": fc1w_pre,
        "fc1b": fc1b_pre,
        "fc2w": fc2w_pre,
        "fc2b": fc2b_pre,
    } for _ in range(NCORES)]
    resC = bass_utils.run_bass_kernel_spmd(ncC, in_mapsC, core_ids=list(range(NCORES)))
    return np.asarray(resC.results[0]["out"], np.float32)


# revision 4
# speedup vs baseline: 1.0082x; 1.0082x over previous
"""Trainium2 Bass kernel v2 for nn_GAT_LSTM: 3-layer GATv2 stack + LSTM + FC head.

Sharding (8 NeuronCores):
  Launch A: data-parallel over the 40 (B,T) graphs -> 5 graphs/core. Edges
            sorted by dst into 126-node blocks. Per chunk of 128 edges:
              S1  PE assembles v = xr[dst] + ee + xl[src] in PSUM. The edge
                  term ee = ea @ We is rank-2, folded into the gather matmul:
                  pne rows 126/127 carry ea0/ea1, xr tile rows 126/127 carry
                  the We rows.  xl[src] comes from bf16 indirect-DMA gathers
                  and is added via an identity matmul.
              S2  ACT prelu -> m;  DVE m*att (2x bf16) + grouped reduce -> lg
              S3  ACT exp at narrow width (20) -> p
              S3' exp writes straight into the pxl tile's den columns
              S4  POOL p*xl broadcast-mult (DVE for 1/16 + all of layer 2),
                  PE one-hot scatter accumulate (den columns ride along).
            The chunk stream is software-pipelined with a 5-stage skew so
            PE/ACT/DVE/POOL run concurrently; ppe/gather loads prefetch two
            blocks ahead (whole-block multi-packet gathers); block tails
            (den/recip -> fp16 PSUM copies -> 2x divide -> bias -> relu) and
            the next layer's transform (ACT-ring DMA transposes, matmuls in
            freed accumulator banks, copies, xld stores) are spread over the
            following ~10 iterations so no engine sees a load spike. The
            layer-0 transform (x @ [Wl0|Wr0]) is host-side input prep.
  Launch B: LSTM w_ih [1024,64000] column-sharded 8-way (bf16), transposed
            matmul (out [128q, 40]) so PE cost is 3x lower; the LSTM biases
            ride along as an extra contraction row on core 0.
  Launch C: partial-gate reduce + LSTM scan + FC head, entirely in the
            transposed layout [128 q-part, ..., 4 batch] - no per-step
            transposes, wide-partition DVE/ACT ops.
"""
import sys

for _p in ("/opt/trn_rl_repo", "/root/.axon_site/_ro/trn_rl_repo"):
    if _p not in sys.path:
        sys.path.insert(0, _p)

from collections import defaultdict

import ml_dtypes
import numpy as np

import concourse.bass as bass
import concourse.bacc as bacc
import concourse.mybir as mybir
import concourse.tile as tile
from concourse import bass_utils
from concourse.masks import make_identity

F32 = mybir.dt.float32
BF16 = mybir.dt.bfloat16
FP16 = mybir.dt.float16
AF = mybir.ActivationFunctionType
OP = mybir.AluOpType
NPBF = ml_dtypes.bfloat16

P = 128
N = 2000
NPAD = 2048
BLK = 126                        # dst nodes per block (126 + 2 ea rows)
NBLK = (N + BLK - 1) // BLK      # 16
B, T = 4, 10
G = B * T                        # 40 graphs
NCORES = 8
GL = G // NCORES                 # 5 graphs per core
LAYERS = [(8, 4, 32), (128, 4, 32), (128, 4, 8)]   # (F_in, H, C)
HID = 256
GATE = 4 * HID                   # 1024
EMB = N * 32                     # 64000
KSL = EMB // NCORES              # 8000 w_ih rows per core
KPAD = ((KSL + 127) // 128) * 128  # 8064
GWS = [640, 640, 256]            # xld gather row widths per layer (padded)

_cache = {}


# ----------------------------------------------------------------------------
# host-side graph preprocessing (indexing + tiny edge-static math)
# ----------------------------------------------------------------------------
def prep_graph(edge_index, edge_attr):
    src = np.concatenate([np.asarray(edge_index[0], np.int64),
                          np.arange(N, dtype=np.int64)])
    dst = np.concatenate([np.asarray(edge_index[1], np.int64),
                          np.arange(N, dtype=np.int64)])
    ea = np.concatenate(
        [edge_attr, np.broadcast_to(np.asarray(edge_attr).mean(0), (N, 2))], axis=0
    ).astype(np.float32)

    order = np.argsort(dst, kind="stable")
    src_s, dst_s, ea_s = src[order], dst[order], ea[order]
    blk_of = dst_s // BLK

    chunks_per_blk = []
    src_pad, ldst_pad, valid_pad, ea_pad = [], [], [], []
    for b in range(NBLK):
        sel = blk_of == b
        e_src = src_s[sel]
        e_ldst = dst_s[sel] - b * BLK
        e_ea = ea_s[sel]
        ne = len(e_src)
        nch = max(1, (ne + P - 1) // P)
        pad = nch * P - ne
        # pad edges: src -> zero row NPAD-1, pne/pen columns all-zero
        src_pad.append(np.concatenate([e_src, np.full(pad, NPAD - 1, np.int64)]))
        ldst_pad.append(np.concatenate([e_ldst, np.zeros(pad, np.int64)]))
        valid_pad.append(np.concatenate([np.ones(ne, bool), np.zeros(pad, bool)]))
        ea_pad.append(np.concatenate([e_ea, np.zeros((pad, 2), np.float32)]))
        chunks_per_blk.append(nch)
    assert min(chunks_per_blk) >= 4, chunks_per_blk

    src_all = np.concatenate(src_pad)
    ldst_all = np.concatenate(ldst_pad)
    valid_all = np.concatenate(valid_pad)
    ea_all = np.concatenate(ea_pad)
    nch_total = sum(chunks_per_blk)

    # dma_gather wrapped-int16 index layout: flat edge i (= j*128+p) lives at
    # idx16[i % 16, i // 16]; replicated across the 8 gpsimd cores (128 rows).
    flat = src_all.astype(np.int16)                                    # [NCH*128]
    idx16 = flat.reshape(-1, 16).T.copy()                              # [16, NCH*8]
    idx16 = np.tile(idx16, (8, 1))                                     # [128, NCH*8]

    jj = np.repeat(np.arange(nch_total), P)
    ee_pos = np.tile(np.arange(P), nch_total)

    # pne_aug [j, n(128), e(128)]: one-hot dst gather + ea rows 126/127
    pne = np.zeros((nch_total, P, P), np.float32)
    pne[jj[valid_all], ldst_all[valid_all], ee_pos[valid_all]] = 1.0
    pne[jj[valid_all], BLK, ee_pos[valid_all]] = ea_all[valid_all, 0]
    pne[jj[valid_all], BLK + 1, ee_pos[valid_all]] = ea_all[valid_all, 1]
    # pen [j, e(128), n(128)]: one-hot scatter
    pen = np.zeros((nch_total, P, P), np.float32)
    pen[jj[valid_all], ee_pos[valid_all], ldst_all[valid_all]] = 1.0

    # merged device layout ppe [128, j, 256]: cols 0:128 = pne [n, j, e],
    # cols 128:256 = pen [e, j, n] (bf16)
    ppe_h = np.concatenate(
        [pne.transpose(1, 0, 2), pen.transpose(1, 0, 2)], axis=2)
    ppe_h = np.ascontiguousarray(ppe_h).astype(NPBF)
    return dict(
        chunks_per_blk=chunks_per_blk, nch_total=nch_total,
        idx16=idx16, ppe_h=ppe_h,
    )


def _bcast_const(vec, reps):
    """[F] -> [128, reps*F] partition-broadcast bf16 constant."""
    t = np.tile(np.asarray(vec, np.float32).reshape(-1), reps)
    return np.ascontiguousarray(np.broadcast_to(t, (P, t.size))).astype(NPBF)


# ----------------------------------------------------------------------------
# Launch A: GAT stack, 5 graphs per core, software-pipelined edge phase
# ----------------------------------------------------------------------------
def build_gat(chunks_per_blk, num_devices=NCORES):
    nch_total = sum(chunks_per_blk)
    maxch = max(chunks_per_blk)
    nc = bacc.Bacc("TRN2", target_bir_lowering=False, debug=False,
                   enable_asserts=False, num_devices=num_devices,
                   dynamic_dma_scratch_size=49152)
    xld0_d = nc.dram_tensor("xld0", [NPAD, GWS[0]], BF16, kind="ExternalInput")
    xr0_d = nc.dram_tensor("xr0", [NBLK * P, 640], BF16, kind="ExternalInput")
    w11_d = nc.dram_tensor("w11", [128, 256], BF16, kind="ExternalInput")
    w21_d = nc.dram_tensor("w21", [128, 64], BF16, kind="ExternalInput")
    ppe_d = nc.dram_tensor("ppe", [P, nch_total, 256], BF16, kind="ExternalInput")
    idx16_d = nc.dram_tensor("idx16", [P, nch_total * 8], mybir.dt.int16,
                             kind="ExternalInput")
    wet_d = [nc.dram_tensor(f"wet{l}", [32, GL * LAYERS[l][1] * LAYERS[l][2]],
                            BF16, kind="ExternalInput") for l in range(3)]
    attb_d = [nc.dram_tensor(f"attb{l}", [P, GL * LAYERS[l][1] * LAYERS[l][2]],
                             BF16, kind="ExternalInput") for l in range(3)]
    biasb_d = [nc.dram_tensor(f"biasb{l}", [P, GL * LAYERS[l][1] * LAYERS[l][2]],
                              FP16, kind="ExternalInput") for l in range(3)]
    emb_d = nc.dram_tensor("emb", [GL, EMB], BF16, kind="ExternalOutput")

    ws_d = [None, w11_d, w21_d]
    # chunk global list: (block, local j, first, last)
    chunk_info = []
    for b in range(NBLK):
        for jl in range(chunks_per_blk[b]):
            chunk_info.append((b, jl, jl == 0, jl == chunks_per_blk[b] - 1))
    NCH = len(chunk_info)
    blk_first_jg = np.cumsum([0] + chunks_per_blk).tolist()

    with tile.TileContext(nc) as tc:
        with (
            tc.tile_pool(name="const", bufs=1) as cp,
            tc.tile_pool(name="xr", bufs=1) as xrp,
            tc.tile_pool(name="oblk", bufs=1) as obp,
            tc.tile_pool(name="gall", bufs=3) as gp,
            tc.tile_pool(name="edge", bufs=6) as ep,
            tc.tile_pool(name="work", bufs=6) as wp,
            tc.tile_pool(name="stage", bufs=2) as stg,
            tc.tile_pool(name="stream", bufs=3) as strm,
            tc.tile_pool(name="psv", bufs=2, space="PSUM") as pv,
            tc.tile_pool(name="psacc", bufs=2, space="PSUM") as pacc,
            tc.tile_pool(name="dram", bufs=1, space="DRAM") as dp,
        ):
            ident = cp.tile([P, P], F32)
            make_identity(nc, ident[:])
            identb = cp.tile([P, P], BF16)
            nc.vector.tensor_copy(out=identb[:], in_=ident[:])
            idx16_t = cp.tile([P, nch_total * 8], mybir.dt.int16)
            nc.sync.dma_start(out=idx16_t[:], in_=idx16_d[:, :])
            zer = cp.tile([P, 640], BF16)
            nc.vector.memset(zer[:], 0.0)
            w_t = [None,
                   cp.tile([128, 256], BF16, tag="w1", name="w1t"),
                   cp.tile([128, 64], BF16, tag="w2", name="w2t")]
            attb_t, biasb_t, wet_t = [], [], []
            for l in range(3):
                if ws_d[l] is not None:
                    nc.sync.dma_start(out=w_t[l][:], in_=ws_d[l][:, :])
                fo = LAYERS[l][1] * LAYERS[l][2]
                at = cp.tile([P, GL * fo], BF16, tag=f"attb{l}", name=f"attb{l}t")
                bt = cp.tile([P, GL * fo], FP16, tag=f"biasb{l}", name=f"biasb{l}t")
                et = cp.tile([32, GL * fo], BF16, tag=f"wet{l}", name=f"wet{l}t")
                nc.sync.dma_start(out=at[:], in_=attb_d[l][:, :])
                nc.sync.dma_start(out=bt[:], in_=biasb_d[l][:, :])
                nc.sync.dma_start(out=et[:], in_=wet_d[l][:, :])
                attb_t.append(at)
                biasb_t.append(bt)
                wet_t.append(et)

            xld = [xld0_d,
                   dp.tile([NPAD, GWS[1]], BF16, name="xld1"),
                   dp.tile([NPAD, GWS[2]], BF16, name="xld2")]
            # zero the pad rows (gathers of pad edges read row NPAD-1);
            # xld0 comes pre-zeroed from the host
            for l in (1, 2):
                nc.sync.dma_start(out=xld[l][NPAD - 32:NPAD, :],
                                  in_=zer[:32, :GWS[l]])

            o_blocks = [obp.tile([P, GL, 128], BF16, tag=f"o{b}", name=f"o{b}")
                        for b in range(NBLK)]
            xr_blocks = [xrp.tile([P, 640], BF16, tag=f"xr{b}", name=f"xr{b}")
                         for b in range(NBLK)]
            # one-time: make o_blocks pad rows finite for the DMA transposes
            for b in range(NBLK):
                nc.vector.memset(o_blocks[b][96:128, :, :], 0.0)

            txf_st = {}

            def txf_mm(l, b, g0, g1, pool_tag):
                """Transform matmuls for graphs [g0, g1) of block b, layer l."""
                fin, hh, cc = LAYERS[l]
                fo = hh * cc
                n0 = b * BLK
                if pool_tag == "acc":
                    ps = pacc.tile([P, 1024], F32, tag="acc")
                else:
                    ps = pv.tile([P, 1024], F32, tag="v")
                if g0 == 0:
                    # We rows for the ee fold (rows 126/127 via 32-aligned
                    # write; rows 96..125 are re-written by the xr copies)
                    nc.vector.tensor_copy(out=xr_blocks[b][96:128, :GL * fo],
                                          in_=wet_t[l][:, :GL * fo])
                hT5 = txf_st[("hT5", b)]
                for g in range(g0, g1):
                    lhsT = hT5[:, g, 0:126]
                    nc.tensor.matmul(
                        out=ps[:126, (g - g0) * 2 * fo:(g - g0 + 1) * 2 * fo],
                        lhsT=lhsT, rhs=w_t[l][:fin, :],
                        start=True, stop=True)
                txf_st[("ps", b, g0)] = ps

            def txf_copy(l, b, g0, g1, eng_xl, eng_xr):
                """PSUM -> xl_st / xr_blocks copies for graphs [g0, g1)."""
                fin, hh, cc = LAYERS[l]
                fo = hh * cc
                ps = txf_st.pop(("ps", b, g0))
                if ("xl", b) not in txf_st:
                    txf_st[("xl", b)] = stg.tile([P, 640], BF16, tag="xl_st",
                                                 name="xl_st")
                xl_st = txf_st[("xl", b)]
                psv = ps[:126, :(g1 - g0) * 2 * fo].rearrange(
                    "p (g t f) -> p g t f", t=2, f=fo)
                eng_xl(out=xl_st[:126, g0 * fo:g1 * fo].rearrange(
                           "p (g f) -> p g f", f=fo), in_=psv[:, :, 0, :])
                eng_xr(out=xr_blocks[b][:126, g0 * fo:g1 * fo].rearrange(
                           "p (g f) -> p g f", f=fo), in_=psv[:, :, 1, :])

            def txf_store(l, b):
                """We rows + xld DRAM writes for block b of layer l."""
                fin, hh, cc = LAYERS[l]
                fo = hh * cc
                gfo = GL * fo
                n0 = b * BLK
                xl_st = txf_st.pop(("xl", b))
                txf_st.pop(("hT5", b), None)
                nc.sync.dma_start(out=xld[l][n0:n0 + 126, :gfo],
                                  in_=xl_st[:126, :gfo])
                if GWS[l] > gfo:
                    nc.sync.dma_start(out=xld[l][n0:n0 + 126, gfo:],
                                      in_=zer[:126, :GWS[l] - gfo])

            def act_copy(out, in_):
                nc.scalar.activation(out=out, in_=in_, func=AF.Copy)

            def txf_transposes(b):
                # ACT HWDGE ring: the wait on o_blocks (an ACT write) is
                # satisfied in ACT program order -> no SP head-of-line block.
                hT5 = stg.tile([P, GL, 128], BF16, tag="hT5", name="hT5")
                for g in range(GL):
                    nc.sync.dma_start(out=hT5[:, g, :],
                                      in_=o_blocks[b][:, g, :],
                                      transpose=True)
                txf_st[("hT5", b)] = hT5

            # ---- layer 0 xr tiles come straight from the host ----
            for b in range(NBLK):
                nc.sync.dma_start(out=xr_blocks[b][:, :],
                                  in_=xr0_d[b * P:(b + 1) * P, :])

            # ---- edge phases, pipelined; transform l+1 folded in ----
            for l in range(3):
                fin, hh, cc = LAYERS[l]
                fo = hh * cc
                gfo = GL * fo
                ghh = GL * hh
                wpx = gfo + ghh
                GW = GWS[l]
                spans = [(s, min(s + 512, gfo)) for s in range(0, gfo, 512)]
                spans_px = [(s, min(s + 512, wpx)) for s in range(0, wpx, 512)]

                ppe_tiles = {}
                gall_tiles = {}
                loaded = [1]
                ps_acc_tiles = {}
                st = {}
                sched = defaultdict(list)

                def load_block(b, l=l, ppe_tiles=ppe_tiles,
                               gall_tiles=gall_tiles, GW=GW):
                    nch = chunks_per_blk[b]
                    j0 = blk_first_jg[b]
                    ppe_t = strm.tile([P, maxch, 256], BF16, tag="ppe")
                    nc.sync.dma_start(out=ppe_t[:, :nch, :],
                                      in_=ppe_d[:, j0:j0 + nch, :])
                    g_all = gp.tile([P, maxch, GW], BF16, tag="gall")
                    nc.gpsimd.dma_gather(
                        out_ap=g_all[:, 0:nch, :],
                        in_ap=xld[l][:, :],
                        idxs_ap=idx16_t[:, j0 * 8:(j0 + nch) * 8],
                        num_idxs=nch * P, num_idxs_reg=nch * P,
                        elem_size=GW, single_packet=False)
                    ppe_tiles[b] = ppe_t
                    gall_tiles[b] = g_all

                def s1(k):
                    b, jl, first, last = chunk_info[k]
                    if first and b == 0:
                        load_block(0)
                    ps_v = pv.tile([P, 1024], F32, tag="v")
                    ppe_t = ppe_tiles[b]
                    g_all = gall_tiles[b]
                    for (s0, s1_) in spans:
                        nc.tensor.matmul(out=ps_v[:, s0:s1_],
                                         lhsT=ppe_t[:, jl, 0:128],
                                         rhs=xr_blocks[b][:, s0:s1_],
                                         start=True, stop=False)
                        nc.tensor.matmul(out=ps_v[:, s0:s1_],
                                         lhsT=identb[:],
                                         rhs=g_all[:, jl, s0:s1_],
                                         start=False, stop=True)
                    st[("v", k)] = ps_v

                def a_prelu(k):
                    ps_v = st.pop(("v", k))
                    m_t = ep.tile([P, 640], BF16, tag="m")
                    nc.scalar.activation(out=m_t[:, :gfo], in_=ps_v[:, :gfo],
                                         func=AF.Prelu, alpha=0.2)
                    st[("m", k)] = m_t

                def v_amtr(k):
                    m_t = st.pop(("m", k))
                    am = wp.tile([P, 640], BF16, tag="am")
                    nc.vector.tensor_tensor(out=am[:, :gfo], in0=m_t[:, :gfo],
                                            in1=attb_t[l][:], op=OP.mult)
                    lg = ep.tile([P, ghh], F32, tag="lg")
                    nc.vector.tensor_reduce(
                        out=lg[:],
                        in_=am[:, :gfo].rearrange("p (t c) -> p t c", c=cc),
                        axis=mybir.AxisListType.X, op=OP.add)
                    st[("lg", k)] = lg

                def a_exp(k):
                    lg = st.pop(("lg", k))
                    # exp written straight into the pxl den columns
                    pxl = ep.tile([P, 680], BF16, tag="pxl", name="pxl")
                    nc.scalar.activation(out=pxl[:, gfo:wpx], in_=lg[:],
                                         func=AF.Exp)
                    st[("pxl", k)] = pxl

                def s_pxl(k):
                    b, jl, first, last = chunk_info[k]
                    if first:
                        while loaded[0] <= b + 2 and loaded[0] < NBLK:
                            load_block(loaded[0])
                            loaded[0] += 1
                    pxl = st[("pxl", k)]
                    g_all = gall_tiles[b]
                    eng = nc.gpsimd if (k % 16 and l < 2) else nc.vector
                    eng.tensor_tensor(
                        out=pxl[:, :gfo].rearrange("p (t c) -> p t c", c=cc),
                        in0=g_all[:, jl, :gfo].rearrange("p (t c) -> p t c", c=cc),
                        in1=pxl[:, gfo:wpx].rearrange("p (t u) -> p t u", u=1)
                            .to_broadcast([P, ghh, cc]),
                        op=OP.mult)

                def s4(k, i):
                    b, jl, first, last = chunk_info[k]
                    pxl = st.pop(("pxl", k))
                    ppe_t = ppe_tiles[b]
                    if first:
                        ps_acc_tiles[b] = pacc.tile([P, 1024], F32, tag="acc",
                                                    name=f"acc{b}")
                    ps_acc = ps_acc_tiles[b]
                    for (s0, s1_) in spans_px:
                        nc.tensor.matmul(out=ps_acc[:, s0:s1_],
                                         lhsT=ppe_t[:, jl, 128:256],
                                         rhs=pxl[:, s0:s1_],
                                         start=first, stop=last)
                    if last:
                        offs = ((1, 1, 1, 2, 2, 2, 3, 3, 4) if b >= NBLK - 2
                                else (2, 3, 4, 5, 6, 7, 8, 9, 10))
                        sched[i + offs[0]].append(lambda b=b: tail_den(b))
                        sched[i + offs[1]].append(lambda b=b: tail_cp(b))
                        sched[i + offs[2]].append(lambda b=b: tail_mult(b))
                        sched[i + offs[3]].append(lambda b=b: tail_bias(b))
                        sched[i + offs[4]].append(lambda b=b: tail_relu(b))
                        if l < 2:
                            sched[i + offs[5]].append(
                                lambda b=b: txf_transposes(b))
                            sched[i + offs[6]].append(
                                lambda b=b: txf_mm(l + 1, b, 0, 4, "acc"))
                            sched[i + offs[7]].append(
                                lambda b=b: (txf_copy(l + 1, b, 0, 4, act_copy,
                                                      act_copy),
                                             txf_mm(l + 1, b, 4, 5, "v")))
                            sched[i + offs[8]].append(
                                lambda b=b: (txf_copy(l + 1, b, 4, 5, act_copy,
                                                      act_copy),
                                             txf_store(l + 1, b)))

                def tail_den(b):
                    ps_acc = ps_acc_tiles[b]
                    den_t = wp.tile([P, ghh], F32, tag="den")
                    nc.vector.tensor_scalar_add(out=den_t[:126, :],
                                                in0=ps_acc[:126, gfo:wpx],
                                                scalar1=1e-4)
                    rec_t = wp.tile([P, ghh], F32, tag="rec")
                    nc.vector.reciprocal(out=rec_t[:126, :], in_=den_t[:126, :])
                    st[("rec", b)] = rec_t

                def tail_cp(b):
                    # PSUM -> bf16 SBUF copies so the divide runs at DVE 2x
                    ps_acc = ps_acc_tiles.pop(b)
                    rec_t = st.pop(("rec", b))
                    acc_sb = wp.tile([P, 640], FP16, tag="acc_sb")
                    nc.scalar.activation(out=acc_sb[:126, :gfo],
                                         in_=ps_acc[:126, :gfo], func=AF.Copy)
                    rec_e = wp.tile([P, 640], FP16, tag="rec_e")
                    nc.scalar.activation(
                        out=rec_e[:126, :gfo].rearrange("p (t c) -> p t c", c=cc),
                        in_=rec_t[:126, :].rearrange("p (t u) -> p t u", u=1)
                            .to_broadcast([126, ghh, cc]),
                        func=AF.Copy)
                    st[("acc", b)] = acc_sb
                    st[("rece", b)] = rec_e

                def tail_mult(b):
                    acc_sb = st.pop(("acc", b))
                    rec_e = st.pop(("rece", b))
                    o_t = wp.tile([P, 640], FP16, tag="o_t")
                    nc.vector.tensor_tensor(out=o_t[:126, :gfo],
                                            in0=acc_sb[:126, :gfo],
                                            in1=rec_e[:126, :gfo], op=OP.mult)
                    st[("ot", b)] = o_t

                def tail_bias(b):
                    o_t = st[("ot", b)]
                    nc.vector.tensor_tensor(out=o_t[:126, :gfo],
                                            in0=o_t[:126, :gfo],
                                            in1=biasb_t[l][:126, :], op=OP.add)

                def tail_relu(b):
                    o_t = st.pop(("ot", b))
                    if l < 2:
                        nc.scalar.activation(
                            out=o_blocks[b][:126, :, :].rearrange(
                                "p g f -> p (g f)"),
                            in_=o_t[:126, :gfo], func=AF.Relu)
                    else:
                        o2 = stg.tile([P, GL, 32], BF16, tag="o2")
                        nc.scalar.activation(
                            out=o2[:126, :, :].rearrange("p g f -> p (g f)"),
                            in_=o_t[:126, :gfo], func=AF.Relu)
                        rows = min(126, N - b * BLK)
                        nc.sync.dma_start(
                            out=emb_d[:, :].rearrange("g (n c) -> g n c", c=32)[
                                :, b * BLK:b * BLK + rows, :]
                                .rearrange("g p c -> p g c"),
                            in_=o2[:rows, :, :])

                # skewed emission: iteration i runs S1(i), exp(i-3), prelu(i-1),
                # am/TR(i-2), pxl(i-4), S4(i-5); block tails + next-layer
                # transforms are spread over iterations i+1 .. i+10.
                for i in range(NCH + 16):
                    if i < NCH:
                        s1(i)
                    if 3 <= i < NCH + 3:
                        a_exp(i - 3)
                    if 1 <= i < NCH + 1:
                        a_prelu(i - 1)
                    if 2 <= i < NCH + 2:
                        v_amtr(i - 2)
                    if 4 <= i < NCH + 4:
                        s_pxl(i - 4)
                    if 5 <= i < NCH + 5:
                        s4(i - 5, i)
                    for fn in sched.pop(i, []):
                        fn()
    nc.compile()
    return nc


# ----------------------------------------------------------------------------
# Launch B: partial LSTM input-gate products, transposed (out [128q, 8, 40])
# ----------------------------------------------------------------------------
def build_gates():
    nc = bacc.Bacc("TRN2", target_bir_lowering=False, debug=False,
                   enable_asserts=False, num_devices=NCORES)
    embT_d = nc.dram_tensor("embT", [KPAD, G], BF16, kind="ExternalInput")
    wT_d = nc.dram_tensor("wT", [KPAD, GATE], BF16, kind="ExternalInput")
    part_d = nc.dram_tensor("part", [P, 8 * G], F32, kind="ExternalOutput")
    KCH = KPAD // P          # 63
    KB = 3                   # k-tiles per DMA chunk (63 = 21*3)
    with tile.TileContext(nc) as tc:
        with (
            tc.tile_pool(name="sb", bufs=1) as sp,
            tc.tile_pool(name="wstream", bufs=3) as wsp,
            tc.tile_pool(name="ps", bufs=1, space="PSUM") as pp,
        ):
            embT_t = sp.tile([P, KCH, G], BF16)
            nc.sync.dma_start(out=embT_t[:],
                              in_=embT_d[:, :].rearrange("(k p) g -> p k g", p=P))
            # one PSUM bank per qt so the 8 k-interleaved accumulation
            # groups live in distinct zero regions
            ps = pp.tile([P, 8, 512], F32)
            for k0 in range(0, KCH, KB):
                w_t = wsp.tile([P, KB, GATE], BF16, tag="w")
                nc.sync.dma_start(
                    out=w_t[:],
                    in_=wT_d[k0 * P:(k0 + KB) * P, :].rearrange(
                        "(k p) q -> p k q", p=P))
                for dk in range(KB):
                    k = k0 + dk
                    for qt in range(8):
                        nc.tensor.matmul(out=ps[:, qt, :G],
                                         lhsT=w_t[:, dk, qt * P:(qt + 1) * P],
                                         rhs=embT_t[:, k, :],
                                         start=(k == 0), stop=(k == KCH - 1))
            out_t = sp.tile([P, 8 * G], F32)
            nc.vector.tensor_copy(out=out_t[:].rearrange("p (a g) -> p a g", g=G),
                                  in_=ps[:, :, :G])
            nc.sync.dma_start(out=part_d[:, :], in_=out_t[:])
    nc.compile()
    return nc


# ----------------------------------------------------------------------------
# Launch C: reduce partials + LSTM scan + FC head (transposed layout)
# ----------------------------------------------------------------------------
def build_scan():
    nc = bacc.Bacc("TRN2", target_bir_lowering=False, debug=False,
                   enable_asserts=False, num_devices=NCORES)
    # parts pre-laid-out host-side to [128, qt(8), g(40), core(8)]
    parts_d = nc.dram_tensor("parts", [P, 8 * G * NCORES], F32,
                             kind="ExternalInput")
    whhT_d = nc.dram_tensor("whhT", [P, 2 * GATE], BF16, kind="ExternalInput")
    fc1w_d = nc.dram_tensor("fc1w", [P, 2 * 512], BF16, kind="ExternalInput")
    fc1b_d = nc.dram_tensor("fc1b", [P, 4], F32, kind="ExternalInput")
    fc2w_d = nc.dram_tensor("fc2w", [P, 4], BF16, kind="ExternalInput")
    fc2b_d = nc.dram_tensor("fc2b", [B, 1], F32, kind="ExternalInput")
    out_d = nc.dram_tensor("out", [B, 1], F32, kind="ExternalOutput")
    with tile.TileContext(nc) as tc:
        with (
            tc.tile_pool(name="sb", bufs=1) as sp,
            tc.tile_pool(name="wk", bufs=2) as wk,
            tc.tile_pool(name="ps", bufs=2, space="PSUM") as pp,
        ):
            parts_t = sp.tile([P, 8 * G, NCORES], F32)
            nc.sync.dma_start(
                out=parts_t[:],
                in_=parts_d[:, :].rearrange("p (q r) -> p q r", r=NCORES))
            whhT_t = sp.tile([P, 2, GATE], BF16)
            nc.sync.dma_start(out=whhT_t[:],
                              in_=whhT_d[:, :].rearrange("p (k q) -> p k q", k=2))
            fc1w_t = sp.tile([P, 2, 512], BF16)
            nc.sync.dma_start(out=fc1w_t[:],
                              in_=fc1w_d[:, :].rearrange("p (k q) -> p k q", k=2))
            fc1b_t = sp.tile([P, 4], F32)
            nc.sync.dma_start(out=fc1b_t[:], in_=fc1b_d[:, :])
            fc2w_t = sp.tile([P, 4], BF16)
            nc.sync.dma_start(out=fc2w_t[:], in_=fc2w_d[:, :])
            fc2b_t = sp.tile([B, 1], F32)
            nc.sync.dma_start(out=fc2b_t[:], in_=fc2b_d[:, :])

            # gihT [128, qt(8), g(40)] = sum over cores (biases folded in B)
            gih_t = sp.tile([P, 8, G], F32)
            nc.vector.tensor_reduce(out=gih_t[:].rearrange("p a g -> p (a g)"),
                                    in_=parts_t[:],
                                    axis=mybir.AxisListType.X, op=OP.add)
            gih_v = gih_t[:].rearrange("p a (g tt) -> p a g tt", tt=T)

            hT = sp.tile([P, 2, B], BF16, tag="h")
            cT = sp.tile([P, 2, B], F32, tag="c")
            nc.vector.memset(hT[:], 0.0)
            nc.vector.memset(cT[:], 0.0)

            for t in range(T):
                ps_g = pp.tile([P, 8, B], F32, tag="g")
                for qt in range(8):
                    for kt in range(2):
                        nc.tensor.matmul(
                            out=ps_g[:, qt, :],
                            lhsT=whhT_t[:, kt, qt * P:(qt + 1) * P],
                            rhs=hT[:, kt, :],
                            start=(kt == 0), stop=(kt == 1))
                g_t = wk.tile([P, 8, B], F32, tag="gt")
                nc.vector.tensor_tensor(
                    out=g_t[:], in0=ps_g[:],
                    in1=gih_v[:, :, :, t], op=OP.add)
                # gate order is host-permuted to [i, f, o, g]
                sif = wk.tile([P, 6, B], F32, tag="sif")
                nc.scalar.activation(out=sif[:], in_=g_t[:, 0:6, :],
                                     func=AF.Sigmoid)
                tg = wk.tile([P, 2, B], F32, tag="tg")
                nc.scalar.activation(out=tg[:], in_=g_t[:, 6:8, :], func=AF.Tanh)
                c_new = sp.tile([P, 2, B], F32, tag=f"c{t}")
                nc.vector.tensor_tensor(out=c_new[:], in0=sif[:, 2:4, :],
                                        in1=cT[:], op=OP.mult)
                it = wk.tile([P, 2, B], F32, tag="it")
                nc.vector.tensor_tensor(out=it[:], in0=sif[:, 0:2, :],
                                        in1=tg[:], op=OP.mult)
                nc.vector.tensor_tensor(out=c_new[:], in0=c_new[:], in1=it[:],
                                        op=OP.add)
                tc_t = wk.tile([P, 2, B], F32, tag="tc")
                nc.scalar.activation(out=tc_t[:], in_=c_new[:], func=AF.Tanh)
                h_new = sp.tile([P, 2, B], BF16, tag=f"h{t}")
                nc.vector.tensor_tensor(out=h_new[:], in0=sif[:, 4:6, :],
                                        in1=tc_t[:], op=OP.mult)
                cT = c_new
                hT = h_new

            lastT = sp.tile([P, 2, B], BF16, tag="lastT")
            nc.scalar.activation(out=lastT[:], in_=hT[:], func=AF.Relu)
            ps_h = pp.tile([P, 4, B], F32, tag="ph")
            for qt in range(4):
                for kt in range(2):
                    nc.tensor.matmul(out=ps_h[:, qt, :],
                                     lhsT=fc1w_t[:, kt, qt * P:(qt + 1) * P],
                                     rhs=lastT[:, kt, :],
                                     start=(kt == 0), stop=(kt == 1))
            hidf = wk.tile([P, 4, B], F32, tag="hidf")
            nc.vector.tensor_tensor(
                out=hidf[:], in0=ps_h[:],
                in1=fc1b_t[:].rearrange("p (q u) -> p q u", u=1)
                    .to_broadcast([P, 4, B]),
                op=OP.add)
            hidT = sp.tile([P, 4, B], BF16, tag="hidT")
            nc.scalar.activation(out=hidT[:], in_=hidf[:], func=AF.Relu)
            ps_o = pp.tile([B, 1], F32, tag="po")
            for kt in range(4):
                nc.tensor.matmul(out=ps_o[:, :], lhsT=hidT[:, kt, :],
                                 rhs=fc2w_t[:, kt:kt + 1], start=(kt == 0),
                                 stop=(kt == 3))
            o_t = wk.tile([B, 1], F32, tag="o")
            nc.vector.tensor_tensor(out=o_t[:], in0=ps_o[:, :], in1=fc2b_t[:],
                                    op=OP.add)
            nc.sync.dma_start(out=out_d[:, :], in_=o_t[:])
    nc.compile()
    return nc


# ----------------------------------------------------------------------------
# kernel entry
# ----------------------------------------------------------------------------
def kernel(**inputs):
    x = np.asarray(inputs["x"], np.float32)
    edge_index = np.asarray(inputs["edge_index"])
    edge_attr = np.asarray(inputs["edge_attr"], np.float32)

    gp = prep_graph(edge_index, edge_attr)
    key = tuple(gp["chunks_per_blk"])
    if ("A", key) not in _cache:
        _cache[("A", key)] = build_gat(gp["chunks_per_blk"])
    if "B" not in _cache:
        _cache["B"] = build_gates()
    if "C" not in _cache:
        _cache["C"] = build_scan()
    ncA, ncB, ncC = _cache[("A", key)], _cache["B"], _cache["C"]

    # ---- Launch A inputs ----
    xg = x.reshape(G, N, 8)
    w01f = np.concatenate([inputs["w_l0"], inputs["w_r0"]], 1).astype(np.float32)
    w11 = np.concatenate([inputs["w_l1"], inputs["w_r1"]], 1).astype(NPBF)
    w21 = np.concatenate([inputs["w_l2"], inputs["w_r2"]], 1).astype(NPBF)
    atts = [inputs["att0"], inputs["att1"], inputs["att2"]]
    biases = [inputs["b0"], inputs["b1"], inputs["b2"]]
    wes = [inputs["w_e0"], inputs["w_e1"], inputs["w_e2"]]
    common = {
        "w11": w11, "w21": w21,
        "ppe": gp["ppe_h"], "idx16": gp["idx16"],
    }
    for l in range(3):
        common[f"attb{l}"] = _bcast_const(atts[l], GL)
        common[f"biasb{l}"] = _bcast_const(biases[l], GL).astype(np.float16)
        fo = LAYERS[l][1] * LAYERS[l][2]
        wet = np.zeros((32, GL * fo), np.float32)
        wet[30:32] = np.tile(np.asarray(wes[l], np.float32), (1, GL))
        common[f"wet{l}"] = np.ascontiguousarray(wet).astype(NPBF)
    in_maps = []
    for c in range(NCORES):
        m = dict(common)
        # layer-0 transform on host: xlr0 [GL, N, 256] = x @ [Wl0 | Wr0]
        xlr0 = xg[c * GL:(c + 1) * GL].astype(np.float32) @ w01f
        xld0 = np.zeros((NPAD, 640), np.float32)
        xld0[:N] = xlr0[:, :, :128].transpose(1, 0, 2).reshape(N, 640)
        xr0 = np.zeros((NBLK * P, 640), np.float32)
        xrn = xlr0[:, :, 128:].transpose(1, 0, 2).reshape(N, 640)
        for b in range(NBLK):
            rows = min(126, N - b * BLK)
            xr0[b * P:b * P + rows] = xrn[b * BLK:b * BLK + rows]
            xr0[b * P + 126:b * P + 128] = np.tile(
                np.asarray(wes[0], np.float32), (1, GL))
        m["xld0"] = xld0.astype(NPBF)
        m["xr0"] = xr0.astype(NPBF)
        in_maps.append(m)
    resA = bass_utils.run_bass_kernel_spmd(ncA, in_maps, core_ids=list(range(NCORES)))
    emb_all = np.concatenate(
        [np.asarray(resA.results[c]["emb"]) for c in range(NCORES)], 0)  # bf16 [G, EMB]

    # ---- Launch B ----
    embT_full = np.ascontiguousarray(emb_all.T)          # [64000, 40] bf16
    # permute LSTM gate order [i, f, g, o] -> [i, f, o, g] so the scan's
    # sigmoids are contiguous
    gperm = np.concatenate([np.arange(0, 512), np.arange(768, 1024),
                            np.arange(512, 768)])
    wT_full = np.ascontiguousarray(
        np.asarray(inputs["w_ih"], np.float32)[gperm].T).astype(NPBF)
    bias_row = (np.asarray(inputs["b_ih"], np.float32)
                + np.asarray(inputs["b_hh"], np.float32))[gperm].astype(NPBF)
    in_mapsB = []
    for c in range(NCORES):
        embT = np.zeros((KPAD, G), NPBF)
        wT = np.zeros((KPAD, GATE), NPBF)
        embT[:KSL] = embT_full[c * KSL:(c + 1) * KSL]
        wT[:KSL] = wT_full[c * KSL:(c + 1) * KSL]
        if c == 0:
            embT[KSL, :] = NPBF(1.0)
            wT[KSL, :] = bias_row
        in_mapsB.append({"embT": embT, "wT": wT})
    resB = bass_utils.run_bass_kernel_spmd(ncB, in_mapsB, core_ids=list(range(NCORES)))
    parts = np.stack([np.asarray(resB.results[c]["part"], np.float32)
                      for c in range(NCORES)], -1)       # [128, 320, 8]

    # ---- Launch C ----
    parts_pre = np.ascontiguousarray(parts.reshape(P, 8 * G * NCORES))
    whhT = np.asarray(inputs["w_hh"], np.float32)[gperm].T  # [256, 1024]
    whhT_pre = np.ascontiguousarray(
        whhT.reshape(2, P, GATE).transpose(1, 0, 2).reshape(P, 2 * GATE)
    ).astype(NPBF)
    fc1w = np.asarray(inputs["fc1_w"], np.float32)       # [256, 512]
    fc1w_pre = np.ascontiguousarray(
        fc1w.reshape(2, P, 512).transpose(1, 0, 2).reshape(P, 2 * 512)
    ).astype(NPBF)
    fc1b_pre = np.ascontiguousarray(
        np.asarray(inputs["fc1_b"], np.float32).reshape(4, P).T)
    fc2w_pre = np.ascontiguousarray(
        np.asarray(inputs["fc2_w"], np.float32).reshape(4, P).T).astype(NPBF)
    fc2b_pre = np.broadcast_to(
        np.asarray(inputs["fc2_b"], np.float32), (B, 1)).copy()
    in_mapsC = [{
        "parts": parts_pre,
        "whhT": whhT_pre,
        "fc1w": fc1w_pre,
        "fc1b": fc1b_pre,
        "fc2w": fc2w_pre,
        "fc2b": fc2b_pre,
    } for _ in range(NCORES)]
    resC = bass_utils.run_bass_kernel_spmd(ncC, in_mapsC, core_ids=list(range(NCORES)))
    return np.asarray(resC.results[0]["out"], np.float32)
